# revision 11
# baseline (speedup 1.0000x reference)
"""MoE QKV parallel linear for Trainium2, 8 NeuronCores.

Problem: out[t] = x[t] @ W[id[t]].T with x [16384, 2048] f32,
W [4, 3072, 2048] f32, id sorted int32 (tokens pre-grouped by expert).

Sharding: tensor-parallel over the QKV output dim (vLLM column-parallel
style). Each core owns a 384-column output shard and streams ALL tokens
through it, so the SPMD program is bit-identical across cores (only the
W slice differs) and expert imbalance costs nothing: ragged token
chunks stream at their actual length on the PE.

Device kernel (per core): W shard resident in SBUF as bf16 stationary
tiles [128k, 128m] reused across 4 consecutive 512-token streams (LDW
amortized 4x), x^T streamed in 2048-token blocks (16 k-tiles each,
double buffered), PE accumulates 16 k-tiles per [128 out, 512 tok]
PSUM bank, DVE copies PSUM->SBUF casting to bf16, out stored
transposed [384, 16384] so every DMA row is contiguous; the host
re-transposes. bf16 throughout: matmul streams 1 col/cycle, DMA and
LDWEIGHTS halve vs fp32, and abs-max rel err stays ~4e-3 (gate 2e-2).
"""

import numpy as np

import concourse.bacc as bacc
import concourse.mybir as mybir
import concourse.tile as tile

NCORES = 8
T = 16384
HIDDEN = 2048
QKV_OUT = 3072
NSH = QKV_OUT // NCORES   # 384 output cols per core
P = 128
KO = HIDDEN // P          # 16 contraction tiles
MT = NSH // P             # 3 stationary out-tiles per core
BLK = 1536                # tokens per x block (3 KB descriptors)
CH = 512                  # tokens per matmul stream (PE max moving dim)

_cache: dict = {}


def _blocks(counts):
    """Compile-time schedule: contiguous expert-pure token blocks.
    The opening blocks ramp 512/1024 so the PE starts before the x
    stream has built its prefetch lead; the global last block is
    capped at 512 so the copy+store tail after the final matmul is
    short. Padding-free: ragged blocks stream their actual length."""
    blocks = []
    off = 0
    first = True
    for e, c in enumerate(counts):
        c = int(c)
        b0 = 0
        while b0 < c:
            if first and b0 == 0 and c > 1536:
                blk = 512
            elif first and b0 == 512 and c > 2048:
                blk = 1024
            else:
                blk = min(BLK, c - b0)
            blocks.append((e, off + b0, blk))
            b0 += blk
        off += c
        first = False
    e, t0, blk = blocks[-1]
    if blk > 512:
        blocks[-1] = (e, t0, blk - 512)
        blocks.append((e, t0 + blk - 512, 512))
    return blocks


def _build(counts):
    """One-core Bass module; identical program on all 8 cores."""
    nc = bacc.Bacc("TRN2", target_bir_lowering=False, debug=False)
    bf16 = mybir.dt.bfloat16
    f32 = mybir.dt.float32

    xT = nc.dram_tensor("xT", [HIDDEN, T], bf16, kind="ExternalInput")
    wT = nc.dram_tensor("wT", [4 * HIDDEN, NSH], bf16, kind="ExternalInput")
    out = nc.dram_tensor("out", [NSH, T], bf16, kind="ExternalOutput")

    blocks = _blocks(counts)
    experts = sorted({e for e, _, _ in blocks})

    # JIT W loads: expert order of first use. The next expert's 16
    # k-tiles are drip-fed 4 per block across the previous expert's
    # early blocks so the one-time W stream never bursts against the
    # x stream during pipeline fill.
    expert_order = []
    for e, _, _ in blocks:
        if e not in expert_order:
            expert_order.append(e)
    w_sched: dict[int, list[tuple[int, int]]] = {}  # block idx -> [(e, ko)]
    w_sched[-1] = [(expert_order[0], ko) for ko in range(KO)]
    for ei in range(1, len(expert_order)):
        prev_e = expert_order[ei - 1]
        prev_blocks = [bi for bi, (e, _, _) in enumerate(blocks) if e == prev_e]
        for j, ko in enumerate(range(KO)):
            slot = prev_blocks[min(j // 4, len(prev_blocks) - 1)]
            w_sched.setdefault(slot, []).append((expert_order[ei], ko))

    with tile.TileContext(nc) as tc:
        with (
            tc.tile_pool(name="wp", bufs=KO * len(experts)) as wp,
            tc.tile_pool(name="xp", bufs=44) as xp,
            tc.tile_pool(name="pp", bufs=8, space="PSUM") as pp,
            tc.tile_pool(name="op", bufs=8) as op,
        ):
            wt = {}

            def load_w(e, ko):
                w = wp.tile([P, NSH], bf16, name=f"w_{e}_{ko}", tag="w")
                nc.scalar.dma_start(
                    out=w[:],
                    in_=wT[(e * KO + ko) * P:(e * KO + ko + 1) * P, :],
                )
                wt[(e, ko)] = w

            for e, ko in w_sched[-1]:
                load_w(e, ko)

            for bi, (e, t0, blk) in enumerate(blocks):
                # Alternate x blocks between the two HWDGE rings: the
                # sequencer's per-descriptor issue cost (~5ns) on a single
                # ring otherwise caps the x stream.
                xeng = nc.sync if bi % 2 == 0 else nc.scalar
                xb = []
                for ko in range(KO):
                    t = xp.tile([P, BLK], bf16, name=f"x_{bi}_{ko}", tag="x")
                    xeng.dma_start(
                        out=t[:, :blk],
                        in_=xT[ko * P:(ko + 1) * P, t0:t0 + blk],
                    )
                    xb.append(t)
                for we, wko in w_sched.get(bi, []):
                    load_w(we, wko)
                chunks = [(c0, min(CH, blk - c0)) for c0 in range(0, blk, CH)]

                for m in range(MT):
                    pss = [None] * len(chunks)
                    for ko in range(KO):
                        lhsT = wt[(e, ko)][:, m * P:(m + 1) * P]
                        for ci, (c0, cn) in enumerate(chunks):
                            if ko == 0:
                                pss[ci] = pp.tile([P, CH], f32,
                                                  name=f"ps_{bi}_{m}_{ci}",
                                                  tag="ps")
                            nc.tensor.matmul(
                                pss[ci][:, :cn], lhsT, xb[ko][:, c0:c0 + cn],
                                start=(ko == 0), stop=(ko == KO - 1),
                            )
                    for ci, (c0, cn) in enumerate(chunks):
                        ot = op.tile([P, CH], bf16, name=f"o_{bi}_{m}_{ci}",
                                     tag="o")
                        nc.vector.tensor_copy(ot[:, :cn], pss[ci][:, :cn])
                        nc.scalar.dma_start(
                            out=out[m * P:(m + 1) * P, t0 + c0:t0 + c0 + cn],
                            in_=ot[:, :cn],
                        )
    nc.compile()
    return nc


def _runner(counts):
    """Compiled 8-core executor, cached by expert counts. Mirrors
    bass2jax.run_bass_via_pjrt's multi-core path (concat per-core
    inputs on axis 0 + shard_map)."""
    import jax
    import jax.numpy as jnp
    from jax.sharding import Mesh, PartitionSpec
    from jax.experimental.shard_map import shard_map
    from concourse import bass2jax, mybir as mb

    nc = _build(counts)
    bass2jax.install_neuronx_cc_hook()

    part_name = nc.partition_id_tensor.name if nc.partition_id_tensor else None
    in_names, out_names, out_avals = [], [], []
    for alloc in nc.m.functions[0].allocations:
        if not isinstance(alloc, mb.MemoryLocationSet):
            continue
        name = alloc.memorylocations[0].name
        if alloc.kind == "ExternalInput":
            if name != part_name:
                in_names.append(name)
        elif alloc.kind == "ExternalOutput":
            out_names.append(name)
            out_avals.append(
                jax.core.ShapedArray(tuple(alloc.tensor_shape),
                                     mb.dt.np(alloc.dtype)))
    n_params = len(in_names)
    n_outs = len(out_names)
    bind_names = in_names + out_names + ([part_name] if part_name else [])

    def _body(*args):
        operands = list(args)
        if part_name:
            operands.append(bass2jax.partition_id_tensor())
        outs = bass2jax._bass_exec_p.bind(
            *operands,
            out_avals=tuple(out_avals),
            in_names=tuple(bind_names),
            out_names=tuple(out_names),
            lowering_input_output_aliases=(),
            sim_require_finite=True,
            sim_require_nnan=True,
            nc=nc,
        )
        return tuple(outs)

    devices = jax.devices()[:NCORES]
    mesh = Mesh(np.asarray(devices), ("core",))
    sharded = jax.jit(
        shard_map(_body, mesh=mesh,
                  in_specs=(PartitionSpec("core"),) * (n_params + n_outs),
                  out_specs=(PartitionSpec("core"),) * n_outs,
                  check_rep=False),
        donate_argnums=tuple(range(n_params, n_params + n_outs)),
        keep_unused=True,
    )

    def run(in_maps):
        concat_in = [
            np.concatenate([m[name] for m in in_maps], axis=0)
            for name in in_names
        ]
        zeros = [np.zeros((NCORES * a.shape[0], *a.shape[1:]), a.dtype)
                 for a in out_avals]
        outs = sharded(*concat_in, *zeros)
        return [
            {name: np.asarray(outs[i]).reshape(NCORES, *out_avals[i].shape)[c]
             for i, name in enumerate(out_names)}
            for c in range(NCORES)
        ]

    return run


def _in_maps(x, W, counts):
    """Host-side shard prep: xT bf16 replicated, W column-shards bf16."""
    import ml_dtypes
    bf16 = ml_dtypes.bfloat16
    xTb = x.T.astype(bf16)  # [2048, 16384], C-contig via astype copy
    maps = []
    for c in range(NCORES):
        wc = W[:, c * NSH:(c + 1) * NSH, :].transpose(0, 2, 1)
        wcb = wc.reshape(4 * HIDDEN, NSH).astype(bf16)
        maps.append({"xT": xTb, "wT": wcb})
    return maps


def kernel(x, W, modality_mapping):
    x = np.ascontiguousarray(np.asarray(x, dtype=np.float32))
    W = np.asarray(W, dtype=np.float32)
    mm = np.asarray(modality_mapping)

    perm = None
    if np.any(np.diff(mm) < 0):  # insurance: tokens not pre-sorted
        perm = np.argsort(mm, kind="stable")
        x = x[perm]
        mm = mm[perm]

    counts = tuple(int(v) for v in
                   np.bincount(mm.astype(np.int64), minlength=W.shape[0]))

    if counts not in _cache:
        _cache[counts] = _runner(counts)
    run = _cache[counts]

    results = run(_in_maps(x, W, counts))

    out = np.empty((T, QKV_OUT), dtype=np.float32)
    for c in range(NCORES):
        out[:, c * NSH:(c + 1) * NSH] = results[c]["out"].T
    if perm is not None:
        inv = np.empty_like(perm)
        inv[perm] = np.arange(T)
        out = out[inv]
    return out
